# revision 1
# baseline (speedup 1.0000x reference)
"""Multi-head attention (B=4, L=2048, D=512, H=8) on 8 Trainium2 cores.

Sharding: core c handles batch b = c//2, query rows [(c%2)*1024, +1024).
K/V projections are split across the two cores sharing a batch (each
projects its own 1024-token half) and exchanged with a pairwise
AllGather through shared DRAM, so attention is fully local afterward.

Device layouts (per core):
  xqT/xkT/xvT (512, 1024)  input slices, transposed (dmodel on partitions)
  qT_all / kT_all          projections kept transposed: head h lives in
                           dmodel-chunk tile h//2 at partition offset 64*(h%2)
  V_sb (128, 520) x16      V natural layout per kv chunk; head h at cols
                           [65h, 65h+64), col 65h+64 = ones (softmax denom)
  scoresT (128kv, 1024q)   PSUM; exp+mask+scale fused into one ACT op
  xsT_ext (65, 512)        PSUM, row 64 = softmax denominator
"""
import numpy as np
import ml_dtypes

import concourse.bacc as bacc
import concourse.bass as bass
import concourse.mybir as mybir
import concourse.tile as tile
from concourse.bass_utils import run_bass_kernel_spmd

F32 = mybir.dt.float32
BF16 = mybir.dt.bfloat16
AF = mybir.ActivationFunctionType

B, L, D = 4, 2048, 512
H, DK = 8, 64
N_CORES = 8
LQ = L // 2            # query rows per core / kv rows projected per core
P = 128
KVC = L // P           # 16 kv chunks
QT = LQ // P           # 8 query tiles of 128
MC = D // P            # 4 dmodel chunks
MASK_BIAS = np.float32(-1e30)

MM_DT = BF16
MM_NP = ml_dtypes.bfloat16 if MM_DT == BF16 else np.float32

_cache = {}


def _build():
    nc = bacc.Bacc("TRN2", target_bir_lowering=False, debug=False,
                   num_devices=N_CORES)

    xqT_d = nc.dram_tensor("xqT", [D, LQ], MM_DT, kind="ExternalInput").ap()
    xkT_d = nc.dram_tensor("xkT", [D, LQ], MM_DT, kind="ExternalInput").ap()
    xvT_d = nc.dram_tensor("xvT", [D, LQ], MM_DT, kind="ExternalInput").ap()
    wq_d = nc.dram_tensor("wq", [D, D], MM_DT, kind="ExternalInput").ap()
    wk_d = nc.dram_tensor("wk", [D, D], MM_DT, kind="ExternalInput").ap()
    wv_d = nc.dram_tensor("wv", [D, D], MM_DT, kind="ExternalInput").ap()
    wo_d = nc.dram_tensor("wo", [D, D], MM_DT, kind="ExternalInput").ap()
    bq_d = nc.dram_tensor("bq", [P, MC], F32, kind="ExternalInput").ap()
    bk_d = nc.dram_tensor("bk", [P, MC], F32, kind="ExternalInput").ap()
    bv_d = nc.dram_tensor("bv", [1, D], MM_DT, kind="ExternalInput").ap()
    bo_d = nc.dram_tensor("bo", [1, D], F32, kind="ExternalInput").ap()
    mb_d = nc.dram_tensor("mb", [P, KVC], F32, kind="ExternalInput").ap()
    out_d = nc.dram_tensor("out", [LQ, D], F32, kind="ExternalOutput").ap()

    PAIRS = [[2 * i, 2 * i + 1] for i in range(N_CORES // 2)]

    with tile.TileContext(nc) as tc:
        with tc.tile_pool(name="const", bufs=1) as cpool, \
             tc.tile_pool(name="xin", bufs=1) as xpool, \
             tc.tile_pool(name="proj", bufs=1) as prpool, \
             tc.tile_pool(name="attn", bufs=17) as apool, \
             tc.tile_pool(name="norm", bufs=4) as npool, \
             tc.tile_pool(name="outp", bufs=3) as opool, \
             tc.tile_pool(name="dram", bufs=1, space="DRAM") as dpool, \
             tc.tile_pool(name="ps", bufs=2, space="PSUM") as ps:

            def load_chunks(pool, ap2d, nm):
                out = []
                for kc in range(MC):
                    t = pool.tile([P, ap2d.shape[1]], ap2d.dtype,
                                  tag=f"{nm}{kc}", name=f"{nm}{kc}")
                    nc.sync.dma_start(t[:], ap2d[kc * P:(kc + 1) * P, :])
                    out.append(t)
                return out

            # interleave weight/input chunk loads in first-use order
            wq = load_chunks(cpool, wq_d, "wq")
            xqT = load_chunks(xpool, xqT_d, "xq")
            bq = cpool.tile_from(bq_d)
            wk = load_chunks(cpool, wk_d, "wk")
            xkT = load_chunks(xpool, xkT_d, "xk")
            bk = cpool.tile_from(bk_d)
            wv = load_chunks(cpool, wv_d, "wv")
            xvT = load_chunks(xpool, xvT_d, "xv")
            wo = load_chunks(cpool, wo_d, "wo")
            bv = cpool.tile_from(bv_d)
            bo = cpool.tile_from(bo_d)
            mb = cpool.tile_from(mb_d)
            ones1 = cpool.tile([1, P], MM_DT)
            nc.vector.memset(ones1[:], 1.0)
            bo_bc = cpool.tile([P, D], F32)
            nc.gpsimd.partition_broadcast(bo_bc[:], bo[:])

            # collective exchange buffers (pairwise AllGather of K/V halves)
            k_own_d = dpool.tile([MC, P, LQ], MM_DT)
            v_own_d = dpool.tile([KVC // 2, P, H * 65], MM_DT)
            k_all_d = dpool.tile([2, MC, P, LQ], MM_DT)
            v_all_d = dpool.tile([2, KVC // 2, P, H * 65], MM_DT)

            # ---- Q projection + own-half K projection (transposed) ----
            qT = [prpool.tile([P, LQ], MM_DT, tag=f"qT{m}", name=f"qT{m}")
                  for m in range(MC)]
            kTo = [prpool.tile([P, LQ], MM_DT, tag=f"kTo{m}", name=f"kTo{m}")
                   for m in range(MC)]
            for m in range(MC):
                for s in range(LQ // 512):
                    pp = ps.tile([P, 512], F32, tag="proj")
                    for kc in range(MC):
                        nc.tensor.matmul(
                            pp[:], wq[kc][:, m * P:(m + 1) * P],
                            xqT[kc][:, s * 512:(s + 1) * 512],
                            start=kc == 0, stop=kc == MC - 1)
                    nc.vector.tensor_scalar_add(qT[m][:, s * 512:(s + 1) * 512],
                                                pp[:], bq[:, m:m + 1])
                for s in range(LQ // 512):
                    pp = ps.tile([P, 512], F32, tag="proj")
                    for kc in range(MC):
                        nc.tensor.matmul(
                            pp[:], wk[kc][:, m * P:(m + 1) * P],
                            xkT[kc][:, s * 512:(s + 1) * 512],
                            start=kc == 0, stop=kc == MC - 1)
                    nc.vector.tensor_scalar_add(kTo[m][:, s * 512:(s + 1) * 512],
                                                pp[:], bk[:, m:m + 1])
                nc.sync.dma_start(k_own_d[m], kTo[m][:])

            # ---- own-half V projection (natural layout + ones columns) ----
            Vo = [prpool.tile([P, H * 65], MM_DT, tag=f"Vo{t}", name=f"Vo{t}")
                  for t in range(KVC // 2)]
            for t in range(KVC // 2):
                pv = ps.tile([P, D], F32, tag="proj")
                for kc in range(MC):
                    nc.tensor.matmul(pv[:], xvT[kc][:, t * P:(t + 1) * P],
                                     wv[kc][:, :], start=kc == 0, stop=False)
                nc.tensor.matmul(pv[:], ones1[0:1, :], bv[0:1, :],
                                 start=False, stop=True)
                vv = Vo[t].rearrange("p (g d) -> p g d", d=65)
                nc.vector.tensor_copy(vv[:, :, 0:64],
                                      pv.rearrange("p (g d) -> p g d", d=64))
                nc.vector.memset(vv[:, :, 64:65], 1.0)
                nc.sync.dma_start(v_own_d[t], Vo[t][:])

            # ---- pairwise K/V exchange ----
            nc.gpsimd.collective_compute(
                "AllGather", mybir.AluOpType.bypass, replica_groups=PAIRS,
                ins=[k_own_d[:]], outs=[k_all_d[:]])
            nc.gpsimd.collective_compute(
                "AllGather", mybir.AluOpType.bypass, replica_groups=PAIRS,
                ins=[v_own_d[:]], outs=[v_all_d[:]])

            kT = [prpool.tile([P, L], MM_DT, tag=f"kT{m}", name=f"kT{m}")
                  for m in range(MC)]
            for m in range(MC):
                for hf in range(2):
                    nc.sync.dma_start(kT[m][:, hf * LQ:(hf + 1) * LQ],
                                      k_all_d[hf, m])
            V = [prpool.tile([P, H * 65], MM_DT, tag=f"V{t}", name=f"V{t}")
                 for t in range(KVC)]
            for t in range(KVC):
                nc.sync.dma_start(V[t][:], v_all_d[t // (KVC // 2),
                                                   t % (KVC // 2)])

            # ---- flash attention per head ----
            xsT2 = [prpool.tile([P, LQ], MM_DT, tag=f"xs{hp}", name=f"xsT2_{hp}")
                    for hp in range(MC)]
            for h in range(H):
                hp, po = h // 2, 64 * (h % 2)
                at = []
                for c in range(KVC):
                    ss = ps.tile([P, 1024], F32, tag="scores", bufs=3)
                    for qh in range(2):
                        nc.tensor.matmul(
                            ss[:, qh * 512:(qh + 1) * 512],
                            kT[hp][po:po + 64, c * P:(c + 1) * P],
                            qT[hp][po:po + 64, qh * 512:(qh + 1) * 512],
                            start=True, stop=True)
                    a = apool.tile([P, 1024], MM_DT, tag="at")
                    nc.scalar.activation(a[:], ss[:], AF.Exp,
                                         bias=mb[:, c:c + 1], scale=0.125)
                    at.append(a)
                xs = [ps.tile([65, 512], F32, tag="proj", name=f"xs_h{h}_{qh}")
                      for qh in range(2)]
                for c in range(KVC):
                    for qh in range(2):
                        nc.tensor.matmul(
                            xs[qh][:], V[c][:, 65 * h:65 * h + 65],
                            at[c][:, qh * 512:(qh + 1) * 512],
                            start=c == 0, stop=c == KVC - 1)
                for qh in range(2):
                    srow = npool.tile([1, 512], F32, tag="srow")
                    nc.vector.tensor_copy(srow[:], xs[qh][64:65, :])
                    rec = npool.tile([1, 512], F32, tag="rec")
                    nc.vector.reciprocal_approx_fast(rec[:], srow[:])
                    bc = npool.tile([64, 512], F32, tag="bc")
                    nc.gpsimd.partition_broadcast(bc[:], rec[:])
                    nc.vector.tensor_mul(
                        xsT2[hp][po:po + 64, qh * 512:(qh + 1) * 512],
                        xs[qh][0:64, :], bc[:])

            # ---- output projection ----
            for qt in range(QT):
                po_ = ps.tile([P, D], F32, tag="proj")
                for hp in range(MC):
                    nc.tensor.matmul(po_[:], xsT2[hp][:, qt * P:(qt + 1) * P],
                                     wo[hp][:, :], start=hp == 0, stop=hp == MC - 1)
                osb = opool.tile([P, D], F32, tag="osb")
                nc.vector.tensor_add(osb[:], po_[:], bo_bc[:])
                nc.sync.dma_start(out_d[qt * P:(qt + 1) * P, :], osb[:])

    nc.compile()
    return nc


def _host_inputs(query, key, value, mask, Wq, bq, Wk, bk, Wv, bv, Wo, bo):
    """Build the 8 per-core input maps (all rank-dependence lives here)."""
    f32 = np.float32
    wq_ = np.ascontiguousarray(Wq).astype(MM_NP)
    wk_ = np.ascontiguousarray(Wk).astype(MM_NP)
    wv_ = np.ascontiguousarray(Wv).astype(MM_NP)
    wo_ = np.ascontiguousarray(Wo).astype(MM_NP)
    bq_ = np.ascontiguousarray(bq.astype(f32).reshape(MC, P).T)
    bk_ = np.ascontiguousarray(bk.astype(f32).reshape(MC, P).T)
    bv_ = bv.astype(MM_NP).reshape(1, D)
    bo_ = bo.astype(f32).reshape(1, D)
    in_maps = []
    for c in range(N_CORES):
        b, half = c // 2, c % 2
        sl = slice(half * LQ, (half + 1) * LQ)
        xqT = np.ascontiguousarray(query[b, sl, :].T).astype(MM_NP)
        xkT = np.ascontiguousarray(key[b, sl, :].T).astype(MM_NP)
        xvT = np.ascontiguousarray(value[b, sl, :].T).astype(MM_NP)
        mbias = np.where(mask[b] == 0, MASK_BIAS, f32(0.0)).astype(f32)
        mb_ = np.ascontiguousarray(mbias.reshape(KVC, P).T)
        in_maps.append({
            "xqT": xqT, "xkT": xkT, "xvT": xvT,
            "wq": wq_, "wk": wk_, "wv": wv_, "wo": wo_,
            "bq": bq_, "bk": bk_, "bv": bv_, "bo": bo_, "mb": mb_,
        })
    return in_maps


def kernel(query, key, value, mask, Wq, bq, Wk, bk, Wv, bv, Wo, bo):
    if "nc" not in _cache:
        _cache["nc"] = _build()
    nc = _cache["nc"]
    in_maps = _host_inputs(query, key, value, mask,
                           Wq, bq, Wk, bk, Wv, bv, Wo, bo)
    res = run_bass_kernel_spmd(nc, in_maps, list(range(N_CORES))).results
    out = np.empty((B, L, D), np.float32)
    for c in range(N_CORES):
        b, half = c // 2, c % 2
        out[b, half * LQ:(half + 1) * LQ, :] = res[c]["out"]
    return out



# revision 5
# speedup vs baseline: 1.1358x; 1.1358x over previous
"""Multi-head attention (B=4, L=2048, D=512, H=8) on 8 Trainium2 cores.

Sharding: core c handles batch b = c//2, query rows [(c%2)*1024, +1024).
Fully local: each core projects the FULL K/V for its batch (no collectives,
no cross-core sync) — costs ~32k extra PE cycles but removes the AllGather
serialization + HAM re-throttle bubbles.

Key optimizations over the exchange-based variant:
  * Score matmuls have K=dk=64, so heads 2i/2i+1 are packed into PE row
    tiles (0,0)/(64,0) and run CONCURRENTLY (2x score throughput). The
    qT/kT layout puts head 2m at partitions 0:64 and 2m+1 at 64:128 of
    dmodel-chunk m, so base_partition auto-derives the tile_position.
  * exp(scores) is split across TWO engines: ACT computes exact Exp for
    even heads; DVE computes a Schraudolph bit-trick exp for odd heads
    (one tensor_scalar: i16 = A16*s + B16, bits reinterpreted as bf16).
    Softmax renormalization cancels the ~3% multiplicative error.
  * Q/K bias+evacuation fused into one ACT activation (per-partition bias).
  * V/O biases folded into the matmul accumulation via K=1 ones-row MMs.
  * V stored with 68-col head groups (64 data + ones col for the softmax
    denominator + pad to keep DVE writes 4B-aligned).

Per-core device layout:
  xqT (512,1024) xkT/xvT (512,2048)  inputs transposed (dmodel on partitions)
  qT (128,1024)x4  kT (128,2048)x4   head h at chunk h//2, partitions 64*(h%2)
  V  (128, 544)x16                   kv chunk tiles; head h cols [68h,68h+64)
                                     data, col 68h+64 = ones (denominator)
  ss (128,1024) PSUM                 scores chunk tile [kv, q] (2 banks)
  xs (65,1024) PSUM                  attnV accum; row 64 = softmax denom
"""
import numpy as np
import ml_dtypes

import concourse.bacc as bacc
import concourse.bass as bass
import concourse.mybir as mybir
import concourse.tile as tile
from concourse.bass_utils import run_bass_kernel_spmd

F32 = mybir.dt.float32
BF16 = mybir.dt.bfloat16
I16 = mybir.dt.int16
AF = mybir.ActivationFunctionType
ALU = mybir.AluOpType

B, L, D = 4, 2048, 512
H, DK = 8, 64
N_CORES = 8
LQ = L // 2            # query rows per core
P = 128
KVC = L // P           # 16 kv chunks
MC = D // P            # 4 dmodel chunks
VW = 68                # per-head stride in V tiles (64 data + ones + pad)
MASK_BIAS = np.float32(-30.0)   # large enough: exp(-30+s) ~ 0

# Schraudolph exp constants (bf16-bits variant): bf16_bits(exp(x)) ~=
# int16(A16*x + B16)
_A = 2.0 ** 23 / np.log(2.0)
_C = 486411.0
A16 = float(_A / 65536.0)
B16 = float((127.0 * 2.0 ** 23 - _C) / 65536.0)

MM_DT = BF16
MM_NP = ml_dtypes.bfloat16

_cache = {}


def _build():
    nc = bacc.Bacc("TRN2", target_bir_lowering=False, debug=False,
                   num_devices=N_CORES)

    xqT_d = nc.dram_tensor("xqT", [D, LQ], MM_DT, kind="ExternalInput").ap()
    xkT_d = nc.dram_tensor("xkT", [D, L], MM_DT, kind="ExternalInput").ap()
    xvT_d = nc.dram_tensor("xvT", [D, L], MM_DT, kind="ExternalInput").ap()
    wq_d = nc.dram_tensor("wq", [D, D], MM_DT, kind="ExternalInput").ap()
    wk_d = nc.dram_tensor("wk", [D, D], MM_DT, kind="ExternalInput").ap()
    wv_d = nc.dram_tensor("wv", [D, D], MM_DT, kind="ExternalInput").ap()
    wo_d = nc.dram_tensor("wo", [D, D], MM_DT, kind="ExternalInput").ap()
    bq_d = nc.dram_tensor("bq", [P, MC], F32, kind="ExternalInput").ap()
    bk_d = nc.dram_tensor("bk", [P, MC], F32, kind="ExternalInput").ap()
    bv_d = nc.dram_tensor("bv", [1, D], MM_DT, kind="ExternalInput").ap()
    bo_d = nc.dram_tensor("bo", [1, D], MM_DT, kind="ExternalInput").ap()
    mb_d = nc.dram_tensor("mb", [P, KVC], F32, kind="ExternalInput").ap()
    b2_d = nc.dram_tensor("b2", [P, KVC], F32, kind="ExternalInput").ap()
    out_d = nc.dram_tensor("out", [LQ, D], F32, kind="ExternalOutput").ap()

    with tile.TileContext(nc) as tc:
        with tc.tile_pool(name="const", bufs=1) as cpool, \
             tc.tile_pool(name="xin", bufs=1) as xpool, \
             tc.tile_pool(name="proj", bufs=1) as prpool, \
             tc.tile_pool(name="attn", bufs=3) as apool, \
             tc.tile_pool(name="norm", bufs=2) as npool, \
             tc.tile_pool(name="outp", bufs=3) as opool, \
             tc.tile_pool(name="ps", bufs=1, space="PSUM") as ps:

            def load_chunks(pool, ap2d, nm):
                out = []
                for kc in range(MC):
                    t = pool.tile([P, ap2d.shape[1]], ap2d.dtype,
                                  tag=f"{nm}{kc}", name=f"{nm}{kc}")
                    nc.sync.dma_start(t[:], ap2d[kc * P:(kc + 1) * P, :])
                    out.append(t)
                return out

            # interleave weight/input chunk loads in first-use order
            wq = load_chunks(cpool, wq_d, "wq")
            xqT = load_chunks(xpool, xqT_d, "xq")
            bq = cpool.tile_from(bq_d)
            wk = load_chunks(cpool, wk_d, "wk")
            xkT = load_chunks(xpool, xkT_d, "xk")
            bk = cpool.tile_from(bk_d)
            wv = load_chunks(cpool, wv_d, "wv")
            xvT = load_chunks(xpool, xvT_d, "xv")
            bv = cpool.tile_from(bv_d)
            wo = load_chunks(cpool, wo_d, "wo")
            bo = cpool.tile_from(bo_d)
            mb = cpool.tile_from(mb_d)
            b2 = cpool.tile_from(b2_d)
            ones1 = cpool.tile([1, P], MM_DT)
            nc.vector.memset(ones1[:], 1.0)

            ss_tag = [0]

            def ps_tile(shape, tag=None):
                if tag is None:
                    tag = "ssA" if ss_tag[0] == 0 else "ssB"
                    ss_tag[0] ^= 1
                return ps.tile(shape, F32, tag=tag, name=f"pp_{tag}")

            # ---- Q projection (8 MMs + 1 ACT bias/evac per m-chunk) ----
            qT = [prpool.tile([P, LQ], MM_DT, tag=f"qT{m}", name=f"qT{m}")
                  for m in range(MC)]
            for m in range(MC):
                pp = ps_tile([P, LQ])
                for s in range(LQ // 512):
                    for kc in range(MC):
                        nc.tensor.matmul(
                            pp[:, s * 512:(s + 1) * 512],
                            wq[kc][:, m * P:(m + 1) * P],
                            xqT[kc][:, s * 512:(s + 1) * 512],
                            start=kc == 0, stop=kc == MC - 1)
                nc.scalar.activation(qT[m][:], pp[:], AF.Identity,
                                     bias=bq[:, m:m + 1])

            # ---- K projection (full batch: 2048 kv rows) ----
            kT = [prpool.tile([P, L], MM_DT, tag=f"kT{m}", name=f"kT{m}")
                  for m in range(MC)]
            for m in range(MC):
                for half in range(2):
                    pp = ps_tile([P, 1024])
                    for s in range(2):
                        for kc in range(MC):
                            nc.tensor.matmul(
                                pp[:, s * 512:(s + 1) * 512],
                                wk[kc][:, m * P:(m + 1) * P],
                                xkT[kc][:, half * 1024 + s * 512:
                                         half * 1024 + (s + 1) * 512],
                                start=kc == 0, stop=kc == MC - 1)
                    nc.scalar.activation(
                        kT[m][:, half * 1024:(half + 1) * 1024], pp[:],
                        AF.Identity, bias=bk[:, m:m + 1])

            # ---- V projection (natural layout, ones col per head group) ----
            v_sb = prpool.tile([P, KVC * VW * H // 1], MM_DT, tag="V",
                               name="v_sb")
            v_g = v_sb.rearrange("p (t h d) -> p t h d", t=KVC, d=VW)
            nc.vector.memset(v_sb[:], 1.0)
            for tp in range(KVC // 2):
                pv = ps_tile([P, 1024])
                for j in range(2):
                    t = 2 * tp + j
                    for kc in range(MC):
                        nc.tensor.matmul(pv[:, j * 512:(j + 1) * 512],
                                         xvT[kc][:, t * P:(t + 1) * P],
                                         wv[kc][:, :], start=kc == 0,
                                         stop=False)
                    nc.tensor.matmul(pv[:, j * 512:(j + 1) * 512],
                                     ones1[0:1, :], bv[0:1, :],
                                     start=False, stop=True)
                nc.vector.tensor_copy(
                    v_g[:, 2 * tp:2 * tp + 2, :, 0:64],
                    pv.rearrange("p (j h d) -> p j h d", j=2, d=64))

            def v_head(t, h):
                return v_g[:, t, h, 0:65]

            # ---- attention: head pairs (2i, 2i+1) with PE row-tile packing ---
            xsT2 = [prpool.tile([P, LQ], MM_DT, tag=f"xs{hp}",
                                name=f"xsT2_{hp}")
                    for hp in range(MC)]
            n_act_extra = {5, 11}   # ssB chunks routed to ACT for balance
            for hp in range(MC):
                hE, hO = 2 * hp, 2 * hp + 1
                xsE = ps.tile([65, LQ], F32, tag="xs", bufs=2, name=f"xsE{hp}")
                xsO = ps.tile([65, LQ], F32, tag="xs", bufs=2, name=f"xsO{hp}")
                at_tiles = {}

                def scores(c):
                    ssA = ps.tile([P, LQ], F32, tag="ssA")
                    ssB = ps.tile([P, LQ], F32, tag="ssB")
                    for qh in range(2):
                        nc.tensor.matmul(
                            ssA[:, qh * 512:(qh + 1) * 512],
                            kT[hp][0:64, c * P:(c + 1) * P],
                            qT[hp][0:64, qh * 512:(qh + 1) * 512],
                            start=True, stop=True)
                        nc.tensor.matmul(
                            ssB[:, qh * 512:(qh + 1) * 512],
                            kT[hp][64:128, c * P:(c + 1) * P],
                            qT[hp][64:128, qh * 512:(qh + 1) * 512],
                            start=True, stop=True)
                    aE = apool.tile([P, LQ], MM_DT, tag="atE", bufs=4)
                    aO = apool.tile([P, LQ], MM_DT, tag="atO", bufs=4)
                    # even head: exact exp on ACT
                    nc.scalar.activation(aE[:], ssA[:], AF.Exp,
                                         bias=mb[:, c:c + 1], scale=0.125)
                    # odd head: Schraudolph exp on DVE (int16 bits == bf16)
                    if c in n_act_extra:
                        nc.scalar.activation(aO[:], ssB[:], AF.Exp,
                                             bias=mb[:, c:c + 1], scale=0.125)
                    else:
                        nc.vector.tensor_scalar(
                            aO.bitcast(I16)[:], ssB[:], A16 * 0.125,
                            b2[:, c:c + 1], ALU.mult, ALU.add)
                    at_tiles[c] = (aE, aO)

                def attnv(c):
                    aE, aO = at_tiles.pop(c)
                    for qh in range(2):
                        nc.tensor.matmul(
                            xsE[:, qh * 512:(qh + 1) * 512], v_head(c, hE),
                            aE[:, qh * 512:(qh + 1) * 512],
                            start=c == 0, stop=c == KVC - 1)
                        nc.tensor.matmul(
                            xsO[:, qh * 512:(qh + 1) * 512], v_head(c, hO),
                            aO[:, qh * 512:(qh + 1) * 512],
                            start=c == 0, stop=c == KVC - 1)

                scores(0)
                scores(1)
                for c in range(2, KVC, 2):
                    scores(c)
                    scores(c + 1)
                    attnv(c - 2)
                    attnv(c - 1)
                attnv(KVC - 2)
                attnv(KVC - 1)

                # normalize: xsT2 = xs[0:64] / xs[64]  (denominator row)
                for par, xs_t in ((0, xsE), (1, xsO)):
                    srow = npool.tile([1, LQ], F32, tag="srow")
                    nc.scalar.copy(srow[:], xs_t[64:65, :])
                    rec = npool.tile([1, LQ], F32, tag="rec")
                    nc.vector.reciprocal_approx_fast(rec[:], srow[:])
                    bc = npool.tile([64, LQ], F32, tag="bc")
                    nc.gpsimd.partition_broadcast(bc[:], rec[:])
                    nc.vector.tensor_tensor(
                        xsT2[hp][64 * par:64 * par + 64, :],
                        xs_t[0:64, :], bc[:], ALU.mult)

            # ---- output projection (bias via K=1 ones MM, ACT evac) ----
            for qt2 in range(LQ // 256):
                po_ = ps_tile([P, 1024])
                for j in range(2):
                    qt = 2 * qt2 + j
                    for hp in range(MC):
                        nc.tensor.matmul(po_[:, j * 512:(j + 1) * 512],
                                         xsT2[hp][:, qt * P:(qt + 1) * P],
                                         wo[hp][:, :], start=hp == 0,
                                         stop=False)
                    nc.tensor.matmul(po_[:, j * 512:(j + 1) * 512],
                                     ones1[0:1, 0:P], bo[0:1, :],
                                     start=False, stop=True)
                osb = opool.tile([P, 1024], F32, tag="osb")
                nc.scalar.copy(osb[:], po_[:])
                for j in range(2):
                    qt = 2 * qt2 + j
                    nc.sync.dma_start(out_d[qt * P:(qt + 1) * P, :],
                                      osb[:, j * 512:(j + 1) * 512])

    nc.compile()
    return nc


def _host_inputs(query, key, value, mask, Wq, bq, Wk, bk, Wv, bv, Wo, bo):
    """Build the 8 per-core input maps (all rank-dependence lives here)."""
    f32 = np.float32
    wq_ = np.ascontiguousarray(Wq).astype(MM_NP)
    wk_ = np.ascontiguousarray(Wk).astype(MM_NP)
    wv_ = np.ascontiguousarray(Wv).astype(MM_NP)
    wo_ = np.ascontiguousarray(Wo).astype(MM_NP)
    bq_ = np.ascontiguousarray(bq.astype(f32).reshape(MC, P).T)
    bk_ = np.ascontiguousarray(bk.astype(f32).reshape(MC, P).T)
    bv_ = bv.astype(MM_NP).reshape(1, D)
    bo_ = bo.astype(MM_NP).reshape(1, D)
    in_maps = []
    for c in range(N_CORES):
        b, half = c // 2, c % 2
        sl = slice(half * LQ, (half + 1) * LQ)
        xqT = np.ascontiguousarray(query[b, sl, :].T).astype(MM_NP)
        xkT = np.ascontiguousarray(key[b].T).astype(MM_NP)
        xvT = np.ascontiguousarray(value[b].T).astype(MM_NP)
        mbias = np.where(mask[b] == 0, MASK_BIAS, f32(0.0)).astype(f32)
        mb_ = np.ascontiguousarray(mbias.reshape(KVC, P).T)
        b2_ = (mb_ * f32(A16) + f32(B16)).astype(f32)
        in_maps.append({
            "xqT": xqT, "xkT": xkT, "xvT": xvT,
            "wq": wq_, "wk": wk_, "wv": wv_, "wo": wo_,
            "bq": bq_, "bk": bk_, "bv": bv_, "bo": bo_,
            "mb": mb_, "b2": b2_,
        })
    return in_maps


def kernel(query, key, value, mask, Wq, bq, Wk, bk, Wv, bv, Wo, bo):
    if "nc" not in _cache:
        _cache["nc"] = _build()
    nc = _cache["nc"]
    in_maps = _host_inputs(query, key, value, mask,
                           Wq, bq, Wk, bk, Wv, bv, Wo, bo)
    res = run_bass_kernel_spmd(nc, in_maps, list(range(N_CORES))).results
    out = np.empty((B, L, D), np.float32)
    for c in range(N_CORES):
        b, half = c // 2, c % 2
        out[b, half * LQ:(half + 1) * LQ, :] = res[c]["out"]
    return out
